# revision 7
# baseline (speedup 1.0000x reference)
"""Trainium2 Bass kernel for CudaTensorProduct (e3nn-style COO tensor product).

Computation: out[b, o] = sum_k cb[k] * in1[b, idx1[k]] * in2[b, idx2[k]]
  in1/in2: (16384, 32) f32, out: (16384, 1024) f32, nnz=4528.

Strategy (per core, pure data-parallel over batch, 2048 rows/core):
  The l-structure factorizes: in1 columns permute into 4 i-sets of 8,
  in2 columns into 2 j-sets of 16.  Every (l1,l2,l3) coupling lives in one
  of 8 pair-tiles q=(a,b) with exactly 128 (i,j) pairs and 128 output rows,
  so the coefficient matrix is block-diagonal with 128x128 blocks:
    U_q  = XI_a * XJ_b                      (DVE mul, bf16 2x mode)
    outT_q = W_q^T @ U_q                    (PE, 512-col grain)

  v4 design notes:
  - Replication slabs for i-slots 0-2 and both j-slots are built on the
    HOST (~2.5MB).  Input DMAs are chunked into 1024-col halves with all
    first-halves heading the bus, so the first DVE mul starts ~11us
    instead of waiting for whole slabs.  Only i-slot 3 is expanded on
    the PE (E-matmul + 2 ACT casts).
  - The PE p-state reaches the full 2.4GHz clock after 3us of continuous
    execution and drops on any idle gap.  Warm-up matmuls ramp it while
    DMAs land, and dependency-free "hotplate" filler matmuls (PSUM
    slices 6,7, never read) are emitted wherever the real stream might
    stall, so the PE never idles.
  - PSUM: slices 0-5 of a flat 128x4096 f32 tensor rotate among real
    matmuls; main tiles are processed as 1024-col half-items (2 slices).
  - Elementwise balance (~15us each): ACT = 2 R casts + 12 out-casts;
    DVE = 16 muls + 4 out-casts (last tiles only -- DVE's FIFO runs the
    muls first, so earlier tiles' casts must clear promptly on ACT).
"""

import os
import sys
import numpy as np
import ml_dtypes

sys.path.insert(0, "/opt/trn_rl_repo")

import concourse.bass as bass
import concourse.mybir as mybir
import concourse.tile as tile
from concourse import bacc
from concourse.bass_utils import run_bass_kernel_spmd

N_CORES = 8
B = 16384
BC = B // N_CORES          # 2048 batch rows per core
D1 = 32
D2 = 32
DOUT = D1 * D2             # 1024
F32 = mybir.dt.float32
BF16 = mybir.dt.bfloat16

LS = [0, 1, 2, 3, 0, 1, 2, 3]


# ----------------------------------------------------------------------------
# Host-side table preprocessing
# ----------------------------------------------------------------------------

def _build_tables(idx1, idx2, out_idx, cb_vals):
    """Build the factorized layout.

    Returns (iperm, e3, w, rows_map):
      iperm: (32,) permutation of in1 columns (iset-major).
      e3: (8, 128) bf16 -- E matrix for i-slot 3; rows index the 8
          permuted-in1 rows 24..31; E[r, p] = 1 iff r == p//16.
      w:  (128, 8*128) bf16 -- w[p, q*128+m] = coefficient for pair p
          (p = i_local*16 + j_local) into scratch out row q*128+m.
      rows_map: (1024,) int -- scratch row -> real out column.
    """
    idx1 = np.asarray(idx1, np.int64)
    idx2 = np.asarray(idx2, np.int64)
    out_idx = np.asarray(out_idx, np.int64)
    cb = np.asarray(cb_vals, np.float64)

    offs, blocks = 0, []
    for l in LS:
        blocks.append(list(range(offs, offs + 2 * l + 1)))
        offs += 2 * l + 1
    isets = [blocks[0] + blocks[3], blocks[1] + blocks[2],
             blocks[4] + blocks[7], blocks[5] + blocks[6]]
    jsets = [list(range(16)), list(range(16, 32))]
    imap = {c: (a, il) for a, s in enumerate(isets) for il, c in enumerate(s)}
    jmap = {c: (b, jl) for b, s in enumerate(jsets) for jl, c in enumerate(s)}

    out_q = {}
    for k in range(len(cb)):
        a, _ = imap[int(idx1[k])]
        b, _ = jmap[int(idx2[k])]
        q = a * 2 + b
        o = int(out_idx[k])
        assert out_q.setdefault(o, q) == q, "coupling crosses pair-tiles"
    rows_map = np.zeros(8 * 128, np.int64)
    out_local = {}
    for q in range(8):
        outs = sorted(o for o, qq in out_q.items() if qq == q)
        assert len(outs) == 128, (q, len(outs))
        for m, o in enumerate(outs):
            out_local[o] = m
            rows_map[q * 128 + m] = o

    e3 = np.zeros((8, 128), np.float32)
    for p in range(128):
        e3[p // 16, p] = 1.0

    w = np.zeros((128, 8 * 128), np.float64)
    for k in range(len(cb)):
        a, il = imap[int(idx1[k])]
        b, jl = jmap[int(idx2[k])]
        q = a * 2 + b
        p = il * 16 + jl
        m = out_local[int(out_idx[k])]
        w[p, q * 128 + m] += cb[k]

    iperm = np.concatenate([np.asarray(s) for s in isets])
    bf = ml_dtypes.bfloat16
    return iperm, e3.astype(bf), w.astype(np.float32).astype(bf), rows_map


# ----------------------------------------------------------------------------
# Device program
# ----------------------------------------------------------------------------

N_WARM = 10          # PE p-state warm-up matmuls (512 cols each)
CH = 512             # matmul column grain
NROT = 6             # rotating PSUM slices (0..5); 6,7 = filler hotplate

# PE emission program after warmup: 'R3' expands i-slot 3 (4x512 + 2 ACT
# casts), ('M', q, h) is main half-item q cols h*1024..(h+1)*1024 (2x512
# + 1 cast + 1 DMA), ('F', n) emits n hotplate fillers.
PE_PROG = [
    ('F', 1), ('R3', 0),
    ('F', 2), ('M', 0, 0), ('F', 1), ('M', 1, 0), ('F', 1), ('M', 2, 0),
    ('F', 1), ('M', 3, 0), ('F', 1), ('M', 4, 0), ('F', 1), ('M', 0, 1),
    ('F', 1), ('M', 1, 1), ('F', 1), ('M', 5, 0), ('F', 1), ('M', 2, 1),
    ('F', 1), ('M', 3, 1), ('F', 1), ('M', 4, 1), ('F', 1), ('M', 5, 1),
    ('F', 1), ('M', 6, 0), ('F', 1), ('M', 7, 0), ('F', 1), ('M', 6, 1),
    ('F', 1), ('M', 7, 1),
]
Q_AB = {q: (q // 2, q % 2) for q in range(8)}
# DVE mul emission order (q, h): host-slab h0s as DMAs land, then h1s,
# then the R3-dependent tiles 6,7.
MUL_ORDER = [(0, 0), (1, 0), (2, 0), (3, 0), (4, 0), (5, 0),
             (0, 1), (1, 1), (2, 1), (3, 1), (4, 1), (5, 1),
             (6, 0), (7, 0), (6, 1), (7, 1)]
# out-cast engine per (q, h): last two tiles ride DVE (after its muls),
# everything else clears promptly on ACT for PSUM slice reuse.
O_CAST = {(q, h): 'a' for q in range(8) for h in range(2)}
O_CAST.update({(6, 0): 'v', (6, 1): 'v', (7, 0): 'v', (7, 1): 'v'})


def _build_bass():
    nc = bacc.Bacc("TRN2", target_bir_lowering=False)

    x3h = nc.dram_tensor("x3h", [8, BC], BF16, kind="ExternalInput")
    xih = nc.dram_tensor("xih", [128, 3 * BC], BF16, kind="ExternalInput")
    xjh = nc.dram_tensor("xjh", [128, 2 * BC], BF16, kind="ExternalInput")
    e3h = nc.dram_tensor("e3h", [8, 128], BF16, kind="ExternalInput")
    wgt = nc.dram_tensor("wgt", [128, 8 * 128], BF16, kind="ExternalInput")
    outT = nc.dram_tensor("outT", [8 * 128, BC], BF16, kind="ExternalOutput")

    with tile.TileContext(nc) as tc:
        with (
            tc.tile_pool(name="work", bufs=1) as work_pool,
            tc.tile_pool(name="ps", bufs=1, space="PSUM") as ps_pool,
        ):
            e3_sb = work_pool.tile([8, 128], BF16)
            w_sb = work_pool.tile([128, 8 * 128], BF16)
            wm_sb = work_pool.tile([128, CH], BF16)

            x3_sb = work_pool.tile([8, BC], BF16)
            xi_sb = work_pool.tile([128, 3 * BC], BF16)   # host i-slabs 0-2
            xj_sb = work_pool.tile([128, 2 * BC], BF16)   # host j-slabs
            r_sb = work_pool.tile([128, BC], BF16)        # PE i-slab 3
            u_sb = work_pool.tile([128, 8 * BC], BF16)    # pair products
            o_sb = work_pool.tile([128, 8 * BC], BF16)    # bf16 out staging

            # ---- input DMAs (1024-col chunks, first halves lead) -------
            H = 1024

            def chunk(t_sb, t_h, slab, h):
                lo = slab * BC + h * H
                return (t_sb[:, lo : lo + H], t_h.ap()[:, lo : lo + H])

            nc.sync.dma_start(out=e3_sb[:], in_=e3h.ap())
            nc.sync.dma_start(out=x3_sb[:], in_=x3h.ap())
            # h0 wave (alternating queues), w mid-bus, then h1 wave
            for args, eng in (
                (chunk(xi_sb, xih, 0, 0), 'a'),
                (chunk(xj_sb, xjh, 0, 0), 's'),
                (chunk(xi_sb, xih, 1, 0), 'a'),
                (chunk(xj_sb, xjh, 1, 0), 's'),
                (chunk(xi_sb, xih, 2, 0), 'a'),
                ((w_sb[:], wgt.ap()), 's'),
                (chunk(xi_sb, xih, 0, 1), 'a'),
                (chunk(xj_sb, xjh, 0, 1), 's'),
                (chunk(xi_sb, xih, 1, 1), 'a'),
                (chunk(xj_sb, xjh, 1, 1), 's'),
                (chunk(xi_sb, xih, 2, 1), 'a'),
            ):
                dst, src = args
                if eng == 'a':
                    nc.scalar.dma_start(out=dst, in_=src)
                else:
                    nc.sync.dma_start(out=dst, in_=src)

            # ---- PE warm-up --------------------------------------------
            nc.gpsimd.memset(wm_sb[:], 0.0)

            ps = ps_pool.tile([128, 8 * CH], F32)
            rot = [0]  # rotation counter over slices 0..NROT-1

            def rot_slice():
                i = rot[0] % NROT
                rot[0] += 1
                return i

            def filler(n):
                for i in range(n):
                    j = 6 + (i % 2)
                    nc.tensor.matmul(
                        ps[:, j * CH : (j + 1) * CH],
                        lhsT=wm_sb[:, :128],
                        rhs=wm_sb[:],
                        start=True,
                        stop=True,
                    )

            filler(N_WARM)

            # ---- main pipeline -----------------------------------------
            def emit_cast(eng, dst, src):
                if eng == 'a':
                    nc.scalar.copy(out=dst, in_=src)
                else:
                    nc.vector.tensor_copy(dst, src)

            def slab(a):
                if a < 3:
                    return xi_sb[:, a * BC : (a + 1) * BC]
                return r_sb

            muls = iter(MUL_ORDER)

            def emit_muls(n):
                for _ in range(n):
                    q, h = next(muls)
                    a, b = Q_AB[q]
                    lo = h * H
                    nc.vector.tensor_mul(
                        u_sb[:, q * BC + lo : q * BC + lo + H],
                        slab(a)[:, lo : lo + H],
                        xj_sb[:, b * BC + lo : b * BC + lo + H],
                    )

            # first 12 muls gate only on host-slab DMA chunks
            emit_muls(12)

            for item in PE_PROG:
                if item[0] == 'F':
                    filler(item[1])
                elif item[0] == 'R3':
                    bases = []
                    for c in range(4):
                        i0 = rot_slice()
                        bases.append(i0)
                        nc.tensor.matmul(
                            ps[:, i0 * CH : (i0 + 1) * CH],
                            lhsT=e3_sb[:],
                            rhs=x3_sb[:, c * CH : (c + 1) * CH],
                            start=True,
                            stop=True,
                        )
                        if c % 2 == 1:
                            h = c // 2
                            i0p = bases[h * 2]
                            assert i0p % 2 == 0 and bases[h * 2 + 1] == i0p + 1
                            emit_cast(
                                'a',
                                r_sb[:, h * H : (h + 1) * H],
                                ps[:, i0p * CH : (i0p + 2) * CH],
                            )
                    emit_muls(4)  # tiles 6,7 (r3-dependent)
                else:
                    _, q, h = item
                    i0 = rot_slice()
                    i1 = rot_slice()
                    assert i1 == i0 + 1 and i0 % 2 == 0, (q, h, i0, i1)
                    for ci in range(2):
                        c = h * 2 + ci
                        nc.tensor.matmul(
                            ps[:, (i0 + ci) * CH : (i0 + ci + 1) * CH],
                            lhsT=w_sb[:, q * 128 : (q + 1) * 128],
                            rhs=u_sb[:, q * BC + c * CH : q * BC + (c + 1) * CH],
                            start=True,
                            stop=True,
                        )
                    dst = o_sb[:, q * BC + h * H : q * BC + (h + 1) * H]
                    emit_cast(O_CAST[(q, h)], dst, ps[:, i0 * CH : (i0 + 2) * CH])
                    nc.sync.dma_start(
                        out=outT.ap()[
                            q * 128 : (q + 1) * 128, h * H : (h + 1) * H
                        ],
                        in_=dst,
                    )

    nc.compile()
    return nc


# ----------------------------------------------------------------------------
# Entry point
# ----------------------------------------------------------------------------

_CACHE = {}


def kernel(in1, in2, cb_vals, idx1, idx2, out_idx):
    in1 = np.ascontiguousarray(np.asarray(in1, np.float32))
    in2 = np.ascontiguousarray(np.asarray(in2, np.float32))

    key = (
        np.asarray(idx1).tobytes(),
        np.asarray(idx2).tobytes(),
        np.asarray(out_idx).tobytes(),
        np.asarray(cb_vals).tobytes(),
    )
    kh = hash(key)
    if kh not in _CACHE:
        iperm, e3, w, rows_map = _build_tables(idx1, idx2, out_idx, cb_vals)
        nc = _build_bass()
        _CACHE[kh] = (nc, iperm, e3, w, rows_map)
    nc, iperm, e3, w, rows_map = _CACHE[kh]

    bf = ml_dtypes.bfloat16
    in1p = in1[:, iperm].astype(bf)
    in2b = in2.astype(bf)
    in_maps = []
    for core in range(N_CORES):
        sl = slice(core * BC, (core + 1) * BC)
        in1T = np.ascontiguousarray(in1p[sl].T)              # (32, BC)
        in2T = np.ascontiguousarray(in2b[sl].T)              # (32, BC)
        x3h = np.ascontiguousarray(in1T[24:32])              # (8, BC)
        # i-slabs 0-2: xi[:, a*BC:...][p, :] = in1T[a*8 + p//16]
        xih = np.ascontiguousarray(np.concatenate(
            [np.repeat(in1T[a * 8 : (a + 1) * 8], 16, axis=0)
             for a in range(3)], axis=1))
        # j-slabs: xj[:, b*BC:...][p, :] = in2T[b*16 + p%16]
        xjh = np.ascontiguousarray(np.concatenate(
            [np.tile(in2T[b * 16 : (b + 1) * 16], (8, 1))
             for b in range(2)], axis=1))
        in_maps.append(
            {"x3h": x3h, "xih": xih, "xjh": xjh, "e3h": e3, "wgt": w}
        )

    trace = bool(int(os.environ.get("KERNEL_TRACE", "0")))
    res = run_bass_kernel_spmd(
        nc, in_maps, core_ids=list(range(N_CORES)), trace=trace
    )
    kernel.last_results = res

    out = np.empty((B, DOUT), np.float32)
    for core in range(N_CORES):
        shard = res.results[core]["outT"]  # (1024, BC) bf16 scratch layout
        out[core * BC : (core + 1) * BC][:, rows_map] = (
            np.asarray(shard).astype(np.float32).T
        )
    return out


# revision 11
# speedup vs baseline: 1.0946x; 1.0946x over previous
"""Trainium2 Bass kernel for CudaTensorProduct (e3nn-style COO tensor product).

Computation: out[b, o] = sum_k cb[k] * in1[b, idx1[k]] * in2[b, idx2[k]]
  in1/in2: (16384, 32) f32, out: (16384, 1024) f32, nnz=4528.

Strategy (per core, pure data-parallel over batch, 2048 rows/core):
  The l-structure factorizes: in1 columns permute into 4 i-sets of 8,
  in2 columns into 2 j-sets of 16.  Every (l1,l2,l3) coupling lives in one
  of 8 pair-tiles q=(a,b) with exactly 128 (i,j) pairs and 128 output rows,
  so the coefficient matrix is block-diagonal with 128x128 blocks:
    U_q  = XI_a * XJ_b                      (DVE mul, bf16 2x mode)
    outT_q = W_q^T @ U_q                    (PE, 512-col grain)

  v4 design notes:
  - Replication slabs for i-slots 0-2 and both j-slots are built on the
    HOST (~2.5MB).  Input DMAs are chunked into 1024-col halves with all
    first-halves heading the bus, so the first DVE mul starts ~11us
    instead of waiting for whole slabs.  Only i-slot 3 is expanded on
    the PE (E-matmul + 2 ACT casts).
  - The PE p-state reaches the full 2.4GHz clock after 3us of continuous
    execution and drops on any idle gap.  Warm-up matmuls ramp it while
    DMAs land, and dependency-free "hotplate" filler matmuls (PSUM
    slices 6,7, never read) are emitted wherever the real stream might
    stall, so the PE never idles.
  - PSUM: slices 0-5 of a flat 128x4096 f32 tensor rotate among real
    matmuls; main tiles are processed as 1024-col half-items (2 slices).
  - Elementwise balance (~15us each): ACT = 2 R casts + 12 out-casts;
    DVE = 16 muls + 4 out-casts (last tiles only -- DVE's FIFO runs the
    muls first, so earlier tiles' casts must clear promptly on ACT).
"""

import os
import sys
import numpy as np
import ml_dtypes

sys.path.insert(0, "/opt/trn_rl_repo")

import concourse.bass as bass
import concourse.mybir as mybir
import concourse.tile as tile
from concourse import bacc
from concourse.bass_utils import run_bass_kernel_spmd

N_CORES = 8
B = 16384
BC = B // N_CORES          # 2048 batch rows per core
D1 = 32
D2 = 32
DOUT = D1 * D2             # 1024
F32 = mybir.dt.float32
BF16 = mybir.dt.bfloat16

LS = [0, 1, 2, 3, 0, 1, 2, 3]


# ----------------------------------------------------------------------------
# Host-side table preprocessing
# ----------------------------------------------------------------------------

def _build_tables(idx1, idx2, out_idx, cb_vals):
    """Build the factorized layout.

    Returns (iperm, e3, w, rows_map):
      iperm: (32,) permutation of in1 columns (iset-major).
      e3: (8, 128) bf16 -- E matrix for i-slot 3; rows index the 8
          permuted-in1 rows 24..31; E[r, p] = 1 iff r == p//16.
      w:  (128, 8*128) bf16 -- w[p, q*128+m] = coefficient for pair p
          (p = i_local*16 + j_local) into scratch out row q*128+m.
      rows_map: (1024,) int -- scratch row -> real out column.
    """
    idx1 = np.asarray(idx1, np.int64)
    idx2 = np.asarray(idx2, np.int64)
    out_idx = np.asarray(out_idx, np.int64)
    cb = np.asarray(cb_vals, np.float64)

    offs, blocks = 0, []
    for l in LS:
        blocks.append(list(range(offs, offs + 2 * l + 1)))
        offs += 2 * l + 1
    isets = [blocks[0] + blocks[3], blocks[1] + blocks[2],
             blocks[4] + blocks[7], blocks[5] + blocks[6]]
    jsets = [list(range(16)), list(range(16, 32))]
    imap = {c: (a, il) for a, s in enumerate(isets) for il, c in enumerate(s)}
    jmap = {c: (b, jl) for b, s in enumerate(jsets) for jl, c in enumerate(s)}

    out_q = {}
    for k in range(len(cb)):
        a, _ = imap[int(idx1[k])]
        b, _ = jmap[int(idx2[k])]
        q = a * 2 + b
        o = int(out_idx[k])
        assert out_q.setdefault(o, q) == q, "coupling crosses pair-tiles"
    rows_map = np.zeros(8 * 128, np.int64)
    out_local = {}
    for q in range(8):
        outs = sorted(o for o, qq in out_q.items() if qq == q)
        assert len(outs) == 128, (q, len(outs))
        for m, o in enumerate(outs):
            out_local[o] = m
            rows_map[q * 128 + m] = o

    e3 = np.zeros((8, 128), np.float32)
    for p in range(128):
        e3[p // 16, p] = 1.0

    w = np.zeros((128, 8 * 128), np.float64)
    for k in range(len(cb)):
        a, il = imap[int(idx1[k])]
        b, jl = jmap[int(idx2[k])]
        q = a * 2 + b
        p = il * 16 + jl
        m = out_local[int(out_idx[k])]
        w[p, q * 128 + m] += cb[k]

    iperm = np.concatenate([np.asarray(s) for s in isets])
    bf = ml_dtypes.bfloat16
    return iperm, e3.astype(bf), w.astype(np.float32).astype(bf), rows_map


# ----------------------------------------------------------------------------
# Device program
# ----------------------------------------------------------------------------

N_WARM = 10          # PE p-state warm-up matmuls (512 cols each)
CH = 512             # matmul column grain
NROT = 6             # rotating PSUM slices (0..5); 6,7 = filler hotplate

# PE emission program after warmup: 'R3' expands i-slot 3 (4x512 + 2 ACT
# casts), ('M', q, h) is main half-item q cols h*1024..(h+1)*1024 (2x512
# + 1 cast + 1 DMA), ('F', n) emits n hotplate fillers.
PE_PROG = [
    ('F', 1), ('R3', 0),
    ('F', 2), ('M', 0, 0), ('F', 1), ('M', 1, 0), ('F', 1), ('M', 2, 0),
    ('F', 1), ('M', 3, 0), ('F', 1), ('M', 6, 0), ('F', 1), ('M', 0, 1),
    ('F', 1), ('M', 7, 0), ('F', 1), ('M', 1, 1), ('F', 1), ('M', 2, 1),
    ('F', 1), ('M', 3, 1), ('F', 1), ('M', 6, 1), ('F', 1), ('M', 7, 1),
    ('F', 1), ('M', 4, 0), ('F', 1), ('M', 5, 0), ('F', 1), ('M', 4, 1),
    ('F', 1), ('M', 5, 1),
]
Q_AB = {q: (q // 2, q % 2) for q in range(8)}
# DVE mul emission order (q, h): host-slab h0s as DMAs land, the
# R3-dependent tiles 6,7 while h1 chunks arrive, and the xi2-fed tiles
# 4,5 last (their slab is at the tail of the DMA bus).
MUL_ORDER = [(0, 0), (1, 0), (2, 0), (3, 0), (6, 0), (0, 1),
             (7, 0), (1, 1), (2, 1), (3, 1), (6, 1), (7, 1),
             (4, 0), (5, 0), (4, 1), (5, 1)]
# out-cast engine per (q, h): the last-consumed tiles (4,5) ride DVE
# (after its muls); everything else clears promptly on ACT so PSUM
# slice reuse never waits on DVE's backlog.
O_CAST = {(q, h): 'a' for q in range(8) for h in range(2)}
O_CAST.update({(4, 0): 'v', (4, 1): 'v', (5, 0): 'v', (5, 1): 'v'})


def _build_bass():
    nc = bacc.Bacc("TRN2", target_bir_lowering=False)

    x3h = nc.dram_tensor("x3h", [8, BC], BF16, kind="ExternalInput")
    xih = nc.dram_tensor("xih", [128, 3 * BC], BF16, kind="ExternalInput")
    xjh = nc.dram_tensor("xjh", [128, 2 * BC], BF16, kind="ExternalInput")
    e3h = nc.dram_tensor("e3h", [8, 128], BF16, kind="ExternalInput")
    wgt = nc.dram_tensor("wgt", [128, 8 * 128], BF16, kind="ExternalInput")
    outT = nc.dram_tensor("outT", [8 * 128, BC], BF16, kind="ExternalOutput")

    with tile.TileContext(nc) as tc:
        with (
            tc.tile_pool(name="work", bufs=1) as work_pool,
            tc.tile_pool(name="ps", bufs=1, space="PSUM") as ps_pool,
        ):
            e3_sb = work_pool.tile([8, 128], BF16)
            w_sb = work_pool.tile([128, 8 * 128], BF16)
            wm_sb = work_pool.tile([128, CH], BF16)

            x3_sb = work_pool.tile([8, BC], BF16)
            xi_sb = work_pool.tile([128, 3 * BC], BF16)   # host i-slabs 0-2
            xj_sb = work_pool.tile([128, 2 * BC], BF16)   # host j-slabs
            r_sb = work_pool.tile([128, BC], BF16)        # PE i-slab 3
            u_sb = work_pool.tile([128, 8 * BC], BF16)    # pair products
            o_sb = work_pool.tile([128, 8 * BC], BF16)    # bf16 out staging

            # ---- input DMAs (1024-col chunks, first halves lead) -------
            H = 1024

            def chunk(t_sb, t_h, slab, h):
                lo = slab * BC + h * H
                return (t_sb[:, lo : lo + H], t_h.ap()[:, lo : lo + H])

            nc.sync.dma_start(out=e3_sb[:], in_=e3h.ap())
            nc.sync.dma_start(out=x3_sb[:], in_=x3h.ap())
            # h0 wave (alternating queues), w mid-bus, h1 wave, xi2 last
            # (its tiles 4,5 are consumed at the end of the pipeline)
            for args, eng in (
                (chunk(xi_sb, xih, 0, 0), 'a'),
                (chunk(xj_sb, xjh, 0, 0), 's'),
                (chunk(xi_sb, xih, 1, 0), 'a'),
                (chunk(xj_sb, xjh, 1, 0), 's'),
                ((w_sb[:], wgt.ap()), 'a'),
                (chunk(xi_sb, xih, 0, 1), 's'),
                (chunk(xj_sb, xjh, 0, 1), 'a'),
                (chunk(xi_sb, xih, 1, 1), 's'),
                (chunk(xj_sb, xjh, 1, 1), 'a'),
                (chunk(xi_sb, xih, 2, 0), 's'),
                (chunk(xi_sb, xih, 2, 1), 'a'),
            ):
                dst, src = args
                if eng == 'a':
                    nc.scalar.dma_start(out=dst, in_=src)
                else:
                    nc.sync.dma_start(out=dst, in_=src)

            # ---- PE warm-up --------------------------------------------
            nc.gpsimd.memset(wm_sb[:], 0.0)

            ps = ps_pool.tile([128, 8 * CH], F32)
            rot = [0]  # rotation counter over slices 0..NROT-1

            def rot_slice():
                i = rot[0] % NROT
                rot[0] += 1
                return i

            def filler(n):
                for i in range(n):
                    j = 6 + (i % 2)
                    nc.tensor.matmul(
                        ps[:, j * CH : (j + 1) * CH],
                        lhsT=wm_sb[:, :128],
                        rhs=wm_sb[:],
                        start=True,
                        stop=True,
                    )

            filler(N_WARM)

            # ---- main pipeline -----------------------------------------
            def emit_cast(eng, dst, src):
                if eng == 'a':
                    nc.scalar.copy(out=dst, in_=src)
                else:
                    nc.vector.tensor_copy(dst, src)

            def slab(a):
                if a < 3:
                    return xi_sb[:, a * BC : (a + 1) * BC]
                return r_sb

            muls = iter(MUL_ORDER)

            def emit_muls(n):
                for _ in range(n):
                    q, h = next(muls)
                    a, b = Q_AB[q]
                    lo = h * H
                    nc.vector.tensor_mul(
                        u_sb[:, q * BC + lo : q * BC + lo + H],
                        slab(a)[:, lo : lo + H],
                        xj_sb[:, b * BC + lo : b * BC + lo + H],
                    )

            # first 4 muls gate only on host-slab DMA chunks; the rest
            # are emitted after the R3 casts exist (tiles 6,7 read r_sb)
            emit_muls(4)

            for item in PE_PROG:
                if item[0] == 'F':
                    filler(item[1])
                elif item[0] == 'R3':
                    bases = []
                    for c in range(4):
                        i0 = rot_slice()
                        bases.append(i0)
                        nc.tensor.matmul(
                            ps[:, i0 * CH : (i0 + 1) * CH],
                            lhsT=e3_sb[:],
                            rhs=x3_sb[:, c * CH : (c + 1) * CH],
                            start=True,
                            stop=True,
                        )
                        if c % 2 == 1:
                            h = c // 2
                            i0p = bases[h * 2]
                            assert i0p % 2 == 0 and bases[h * 2 + 1] == i0p + 1
                            emit_cast(
                                'a',
                                r_sb[:, h * H : (h + 1) * H],
                                ps[:, i0p * CH : (i0p + 2) * CH],
                            )
                    emit_muls(12)  # remaining muls (incl. r3-dependent)
                else:
                    _, q, h = item
                    i0 = rot_slice()
                    i1 = rot_slice()
                    assert i1 == i0 + 1 and i0 % 2 == 0, (q, h, i0, i1)
                    for ci in range(2):
                        c = h * 2 + ci
                        nc.tensor.matmul(
                            ps[:, (i0 + ci) * CH : (i0 + ci + 1) * CH],
                            lhsT=w_sb[:, q * 128 : (q + 1) * 128],
                            rhs=u_sb[:, q * BC + c * CH : q * BC + (c + 1) * CH],
                            start=True,
                            stop=True,
                        )
                    dst = o_sb[:, q * BC + h * H : q * BC + (h + 1) * H]
                    emit_cast(O_CAST[(q, h)], dst, ps[:, i0 * CH : (i0 + 2) * CH])
                    nc.sync.dma_start(
                        out=outT.ap()[
                            q * 128 : (q + 1) * 128, h * H : (h + 1) * H
                        ],
                        in_=dst,
                    )

    nc.compile()
    return nc


# ----------------------------------------------------------------------------
# Entry point
# ----------------------------------------------------------------------------

_CACHE = {}


def kernel(in1, in2, cb_vals, idx1, idx2, out_idx):
    in1 = np.ascontiguousarray(np.asarray(in1, np.float32))
    in2 = np.ascontiguousarray(np.asarray(in2, np.float32))

    key = (
        np.asarray(idx1).tobytes(),
        np.asarray(idx2).tobytes(),
        np.asarray(out_idx).tobytes(),
        np.asarray(cb_vals).tobytes(),
    )
    kh = hash(key)
    if kh not in _CACHE:
        iperm, e3, w, rows_map = _build_tables(idx1, idx2, out_idx, cb_vals)
        nc = _build_bass()
        _CACHE[kh] = (nc, iperm, e3, w, rows_map)
    nc, iperm, e3, w, rows_map = _CACHE[kh]

    bf = ml_dtypes.bfloat16
    in1p = in1[:, iperm].astype(bf)
    in2b = in2.astype(bf)
    in_maps = []
    for core in range(N_CORES):
        sl = slice(core * BC, (core + 1) * BC)
        in1T = np.ascontiguousarray(in1p[sl].T)              # (32, BC)
        in2T = np.ascontiguousarray(in2b[sl].T)              # (32, BC)
        x3h = np.ascontiguousarray(in1T[24:32])              # (8, BC)
        # i-slabs 0-2: xi[:, a*BC:...][p, :] = in1T[a*8 + p//16]
        xih = np.ascontiguousarray(np.concatenate(
            [np.repeat(in1T[a * 8 : (a + 1) * 8], 16, axis=0)
             for a in range(3)], axis=1))
        # j-slabs: xj[:, b*BC:...][p, :] = in2T[b*16 + p%16]
        xjh = np.ascontiguousarray(np.concatenate(
            [np.tile(in2T[b * 16 : (b + 1) * 16], (8, 1))
             for b in range(2)], axis=1))
        in_maps.append(
            {"x3h": x3h, "xih": xih, "xjh": xjh, "e3h": e3, "wgt": w}
        )

    trace = bool(int(os.environ.get("KERNEL_TRACE", "0")))
    res = run_bass_kernel_spmd(
        nc, in_maps, core_ids=list(range(N_CORES)), trace=trace
    )
    kernel.last_results = res

    out = np.empty((B, DOUT), np.float32)
    for core in range(N_CORES):
        shard = res.results[core]["outT"]  # (1024, BC) bf16 scratch layout
        out[core * BC : (core + 1) * BC][:, rows_map] = (
            np.asarray(shard).astype(np.float32).T
        )
    return out
